# revision 15
# baseline (speedup 1.0000x reference)
"""Trainium2 Bass kernel for nn_Attention3d_9483287790337.

Math: 1x1x1-conv QKV -> per-head (softmax_d q * scale) @ (softmax_n k) attention
over n=4096 tokens -> out proj -> channel LayerNorm.

Key numerical fact exploited: k's softmax is over the 4096 tokens, so k-tilde
entries are ~2.4e-4 and sim = q~^T k~ lies in [0, ~1.6e-4]. exp(sim) is linear
to ~8 significant digits (error ~sim^2/2 ~ 1e-8 relative), so
  attn = softmax(sim) = (1 + sim) / (n + SCALE)      [denominator is constant:
                                                      sum_j sim_ij = SCALE]
and the attention output factorizes through associativity:
  out = (vsum + B^T q~s) / (n*(n+SCALE)),  B[d',d] = sum_j ek~T[j,d'] vT[j,d]
This reduces the device work to ~n*d^2 instead of n^2*d.

Perf structure (vs the first working version):
  - All free-dim>=256 matmuls run as float32r (same bits, 1 cycle/row on the
    PE vs 4 for float32).
  - ekT / vT are stored bf16; the 64 B-accumulation matmuls (free dim 129)
    run in bf16 at 1 cycle/row.
  - rsk (column sums of ekT) is folded into the B matmul as a ones column in
    the vT operand, eliminating 32 column-sum matmuls and a DRAM roundtrip.
  - vsum falls out of a 3rd accumulating ones-matmul over vT.
  - kT and vT projections share one PSUM chunk loop so the stationary x
    blocks stay hot.

Sharding: 8-way data parallel over tokens (512 tokens/core); k/v/B work is
replicated per core (no cross-device comms). Weights replicated.
"""

import numpy as np
from contextlib import ExitStack

import concourse.bass as bass
import concourse.tile as tile
from concourse import mybir
import orjson

F32 = mybir.dt.float32
F32R = mybir.dt.float32r
BF16 = mybir.dt.bfloat16
AX = mybir.AxisListType
OP = mybir.AluOpType
AF = mybir.ActivationFunctionType

DIM = 256
HEADS = 4
DHEAD = 64
N = 4096           # tokens
TOK = 512          # tokens per core
NCORES = 8
SCALE = DHEAD ** -0.5
NORM = 1.0 / (N * (N + SCALE))
NJB = N // 128     # 32 j-blocks
CHUNK = 2          # j-blocks per psum chunk in the kT/vT projections


def _r(ap):
    return ap.bitcast(F32R)


# --------------------------------------------------------------------------
# Workaround for this container's walrus build: its ISA encoding accepts at
# most ONE sync-wait per instruction, but tile.py emits several `on_wait`
# entries on one instruction. Split extras into single-wait NoOps on the same
# engine (engines execute their stream in order, so sequential waits are
# equivalent).
# --------------------------------------------------------------------------

_ENGINES = {"Pool", "Activation", "PE", "DVE", "SP"}
_SPLIT_OPCODE = "Drain"


def _split_multi_waits(bir_bytes: bytes) -> bytes:
    m = orjson.loads(bir_bytes)

    def walk(block):
        ins = block.get("instructions")
        if ins:
            out = []
            for inst in ins:
                si = inst.get("sync_info")
                waits = (si or {}).get("on_wait") or []
                if len(waits) > 1 and inst.get("engine") in _ENGINES:
                    for j, w in enumerate(waits[:-1]):
                        out.append({
                            "engine": inst["engine"],
                            "ins": [],
                            "outs": [],
                            "name": f"{inst.get('name', 'i')}_sw{j}",
                            "opcode": _SPLIT_OPCODE,
                            "sync_info": {"on_update": [], "on_wait": [w]},
                        })
                    si["on_wait"] = [waits[-1]]
                out.append(inst)
            block["instructions"] = out
        for sub in block.get("blocks") or []:
            walk(sub)

    for fn in m["functions"]:
        for b in fn["blocks"]:
            walk(b)
    return orjson.dumps(m)


_fix_installed = False


def _install_bir_fix():
    global _fix_installed
    if _fix_installed:
        return
    _fix_installed = True
    import concourse.bass_utils as bu
    import concourse.bass2jax as b2j

    orig = bu.compile_bir_kernel

    def patched(bir_json, tmpdir, neff_name="file.neff"):
        return orig(_split_multi_waits(bir_json), tmpdir, neff_name=neff_name)

    bu.compile_bir_kernel = patched
    b2j.compile_bir_kernel = patched


# --------------------------------------------------------------------------
# Device kernel
# --------------------------------------------------------------------------

def _make_pools(tc, ctx):
    const = ctx.enter_context(tc.tile_pool(name="const", bufs=1))
    sb = ctx.enter_context(tc.tile_pool(name="sb", bufs=1))
    wk = ctx.enter_context(tc.tile_pool(name="wk", bufs=8))
    pp = ctx.enter_context(tc.tile_pool(name="pp", bufs=4, space="PSUM"))
    pacc = ctx.enter_context(tc.tile_pool(name="pacc", bufs=1, space="PSUM"))
    return const, sb, wk, pp, pacc


def _emit(nc, tc, ctx, t, pools=None):
    if pools is None:
        pools = _make_pools(tc, ctx)
    const, sb, wk, pp, pacc = pools

    # ---- constants / inputs to SBUF.
    # DMA queue plan: sync carries wall-half A then x half 0; gpsimd carries
    # wall-half B then x half 1; scalar carries the small constants. This gets
    # the first projection chunk running by ~7us instead of ~22us.
    # wall layout: [wkv0 (wk0|wv0), wkv1, wq0, wq1, wo0, wo1]
    wall = const.tile([128, 8, 256], F32R, name="wall", tag="wall")
    nc.sync.dma_start(out=wall[:, 0:4, :], in_=t["wall"][:, 0:4, :])
    nc.gpsimd.dma_start(out=wall[:, 4:8, :], in_=t["wall"][:, 4:8, :])
    wkv_sb = [wall[:, 0:2, :], wall[:, 2:4, :]]
    wq_sb = [wall[:, 4, :], wall[:, 5, :]]
    wo_sb = [wall[:, 6, :], wall[:, 7, :]]

    x_sb, xt_sb = [], []
    for b in range(2):
        xtb = sb.tile([128, TOK], F32R, name=f"xt{b}", tag=f"xt{b}")
        nc.scalar.dma_start(out=xtb, in_=t["xt"][b * 128:(b + 1) * 128, :])
        xt_sb.append(xtb)
    for b in range(2):
        xb = sb.tile([128, N], F32R, name=f"x{b}", tag=f"x{b}")
        eng = nc.sync if b == 0 else nc.gpsimd
        for q in range(4):
            eng.dma_start(out=xb[:, q * 1024:(q + 1) * 1024],
                          in_=t["xf"][b * 128:(b + 1) * 128,
                                      q * 1024:(q + 1) * 1024])
        x_sb.append(xb)

    ones_col2 = const.tile([128, 2], F32R, name="ones_col2", tag="ones_col2")
    nc.scalar.dma_start(out=ones_col2, in_=t["oc2"][:, :])
    ind2_sb = const.tile([2, 128], F32R, name="ind2_sb", tag="ind2_sb")
    nc.scalar.dma_start(out=ind2_sb, in_=t["ind2"][:, :])

    g_bc = const.tile([128, 256], F32, name="g_bc", tag="g_bc")
    ap = t["g"][:]
    bcast = bass.AP(tensor=ap.tensor, offset=ap.offset,
                    ap=[[0, 128]] + list(ap.ap))
    nc.scalar.dma_start(out=g_bc, in_=bcast)
    onesr_sb = const.tile([1, 128], F32R, name="onesr_sb", tag="onesr_sb")
    nc.scalar.dma_start(out=onesr_sb, in_=t["onesr"][:, :])
    br_sb = const.tile([1, 256], F32R, name="br_sb", tag="br_sb")
    nc.scalar.dma_start(out=br_sb, in_=t["br"][:, :])

    ones_bf = const.tile([128, 1], BF16, name="ones_bf", tag="ones_bf")
    nc.vector.memset(ones_bf, 1.0)
    ones_f1 = const.tile([1, 1], F32, name="ones_f1", tag="ones_f1")
    nc.vector.memset(ones_f1, 1.0)
    eps_t = const.tile([128, 1], F32, name="eps_t", tag="eps_t")
    nc.vector.memset(eps_t, 1e-5)

    # ---- token-major ekT / vT stores (bf16). vT layout per j-block:
    # [one, v_cb0 (128), one, v_cb1 (128)] so each cb's B matmul rhs is the
    # contiguous 129 columns [ones | v_half], yielding rsk in pB column 0.
    ekT = sb.tile([128, NJB, 256], BF16, name="ekT", tag="ekT")
    vt = sb.tile([128, NJB, 258], BF16, name="vt", tag="vt")
    nc.vector.memset(vt[:, :, 0:1], 1.0)
    nc.vector.memset(vt[:, :, 129:130], 1.0)

    # ---- q path: q = w_q @ xt ; eq = exp(q); per-head rowsums over d
    eq_sb, rcp2_sb = [], []
    for cb in range(2):
        pq = pp.tile([128, TOK], F32, name=f"pq{cb}", tag="p")
        for inb in range(2):
            nc.tensor.matmul(pq, wq_sb[inb][:, cb * 128:(cb + 1) * 128],
                             xt_sb[inb], start=(inb == 0), stop=(inb == 1))
        eq = sb.tile([128, TOK], F32R, name=f"eq{cb}", tag=f"eq{cb}")
        nc.scalar.activation(out=eq, in_=pq, func=AF.Exp)
        eq_sb.append(eq)
        # per-head-half column sums over the 128 partitions -> [2, TOK]
        prs = pp.tile([2, TOK], F32, name=f"prs{cb}", tag="p")
        nc.tensor.matmul(prs, ones_col2, eq, start=True, stop=True)
        # 1/x as exp(-ln(x)) on ACT (custom-DVE recip ops don't lower in
        # this walrus build; DVE iterative divide is slow on 2 partitions)
        lnp = const.tile([2, TOK], F32, name=f"lnp{cb}", tag=f"lnp{cb}")
        nc.scalar.activation(out=lnp, in_=prs, func=AF.Ln)
        rcp2 = const.tile([2, TOK], F32R, name=f"rcp2{cb}", tag=f"rcp2{cb}")
        nc.scalar.activation(out=rcp2, in_=lnp, func=AF.Exp, scale=-1.0)
        rcp2_sb.append(rcp2)

    # ---- fused kT/vT projections + B/rsk/vsum accumulation
    # pB[cb][d', 0] = rsk over cb's channels; pB[cb][d', 1+d] = B[d', d].
    # pvs[0, 1+129*cb+d] = vsum[d] for cb's channels.
    pB = [pacc.tile([128, 132], F32, name=f"pB{cb}", tag=f"accB{cb}")
          for cb in range(2)]
    B_bd = [sb.tile([128, 128], F32R, name=f"Bbd{cb}", tag=f"Bbd{cb}")
            for cb in range(2)]
    for cb in range(2):
        nc.scalar.dma_start(out=B_bd[cb], in_=t["bz"][:, :])
    pvs = pacc.tile([1, 258], F32, name="pvs", tag="accv")
    for jb in range(NJB):
        pkv = pp.tile([128, 2, 256], F32, name=f"pkv{jb}", tag="p")
        for inb in range(2):
            nc.tensor.matmul(pkv, x_sb[inb][:, jb * 128:(jb + 1) * 128],
                             wkv_sb[inb], start=(inb == 0), stop=(inb == 1))
        nc.scalar.activation(out=ekT[:, jb, :], in_=pkv[:, 0, :], func=AF.Exp)
        # copy v -> vt halves (cast to bf16) in one strided DVE op; the
        # out AP jumps 129 cols between halves to skip the ones columns
        vslice = bass.AP(tensor=vt.tensor, offset=vt.offset + jb * 258 + 1,
                         ap=[list(vt.ap[0]), [129, 2], [1, 128]])
        nc.vector.tensor_copy(vslice, pkv[:, 1, :])
        for cb in range(2):
            nc.tensor.matmul(pB[cb][:, 0:129],
                             ekT[:, jb, cb * 128:(cb + 1) * 128],
                             vt[:, jb, cb * 129:cb * 129 + 129],
                             start=(jb == 0), stop=(jb == NJB - 1))
        nc.tensor.matmul(pvs, ones_bf, vt[:, jb, :],
                         start=(jb == 0), stop=(jb == NJB - 1))

    # ---- rk (1/rsk), B blocks, vsum to partition-major
    rk_p = const.tile([128, 2], F32, name="rk_p", tag="rk_p")
    for cb in range(2):
        nc.vector.reciprocal(rk_p[:, cb:cb + 1], pB[cb][:, 0:1])
        for hp in range(2):
            r0 = 64 * hp
            nc.vector.tensor_copy(B_bd[cb][r0:r0 + 64, r0:r0 + 64],
                                  pB[cb][r0:r0 + 64, 1 + r0:1 + r0 + 64])
    pvs_row = const.tile([1, 258], F32, name="pvs_row", tag="pvs_row")
    nc.vector.tensor_copy(pvs_row, pvs)
    pvT = pacc.tile([128, 2], F32, name="pvT", tag="accv2")
    for cb in range(2):
        # [1,128] free-major -> [128,1] partition-major via PE transpose
        nc.tensor.transpose(pvT[:, cb:cb + 1],
                            pvs_row[0:1, 1 + 129 * cb:129 + 129 * cb],
                            ones_f1)

    # ---- q~s = eq * rk[d'] * (SCALE * rcp2[half(d'), i]) ; numer ; out_all
    out_all = []
    for cb in range(2):
        qsc = pp.tile([128, TOK], F32, name=f"qsc{cb}", tag="p")
        nc.tensor.matmul(qsc, ind2_sb, rcp2_sb[cb], start=True,
                         stop=True)
        qs = sb.tile([128, TOK], F32R, name=f"qs{cb}", tag=f"qs{cb}")
        pnum = pp.tile([128, TOK], F32, name=f"pnum{cb}", tag="p")
        oa = sb.tile([128, TOK], F32R, name=f"oall{cb}", tag=f"oall{cb}")
        for th in range(2):
            sl = slice(th * 256, (th + 1) * 256)
            nc.vector.scalar_tensor_tensor(out=qs[:, sl],
                                           in0=eq_sb[cb].bitcast(F32)[:, sl],
                                           scalar=rk_p[:, cb:cb + 1],
                                           in1=qsc[:, sl],
                                           op0=OP.mult, op1=OP.mult)
            nc.tensor.matmul(pnum[:, sl], B_bd[cb], qs[:, sl], start=True,
                             stop=True)
            nc.vector.tensor_scalar(out=oa[:, sl], in0=pnum[:, sl],
                                    scalar1=pvT[:, cb:cb + 1], scalar2=NORM,
                                    op0=OP.add, op1=OP.mult)
        out_all.append(oa)

    # ---- y = w_out @ out + b_out (as yT [t, o]), then channel LayerNorm
    for tb in range(TOK // 128):
        py = pp.tile([128, 256], F32, name=f"py{tb}", tag="p")
        for cb in range(2):
            nc.tensor.matmul(py, out_all[cb][:, tb * 128:(tb + 1) * 128],
                             wo_sb[cb], start=(cb == 0), stop=False)
        # bias add as a K=1 accumulating matmul: py += ones^T @ b_row
        nc.tensor.matmul(py, onesr_sb, br_sb, start=False, stop=True)
        stats = wk.tile([128, 6], F32, name=f"st{tb}", tag="w_small")
        nc.vector.bn_stats(out=stats, in_=py)
        mv = wk.tile([128, 2], F32, name=f"mv{tb}", tag="w_small")
        nc.vector.bn_aggr(out=mv, in_=stats)
        lnv = wk.tile([128, 1], F32, name=f"lv{tb}", tag="w_small")
        nc.scalar.activation(out=lnv, in_=mv[:, 1:2], func=AF.Ln, bias=eps_t)
        rstd = wk.tile([128, 1], F32, name=f"rs{tb}", tag="w_small")
        nc.scalar.activation(out=rstd, in_=lnv, func=AF.Exp, scale=-0.5)
        yn = wk.tile([128, 256], F32, name=f"yn{tb}", tag="w_yb")
        nc.vector.tensor_scalar(out=yn, in0=py, scalar1=mv[:, 0:1],
                                scalar2=rstd, op0=OP.subtract, op1=OP.mult)
        yo = wk.tile([128, 256], F32, name=f"yo{tb}", tag="w_yb")
        nc.gpsimd.tensor_mul(out=yo, in0=yn, in1=g_bc)
        e0, e1 = ((nc.sync, nc.gpsimd), (nc.scalar, nc.sync),
                  (nc.gpsimd, nc.scalar), (nc.sync, nc.gpsimd))[tb]
        e0.dma_start(out=t["yt"][tb * 128:(tb + 1) * 128, 0:128],
                     in_=yo[:, 0:128])
        e1.dma_start(out=t["yt"][tb * 128:(tb + 1) * 128, 128:256],
                     in_=yo[:, 128:256])


def build_nc(niter=1):
    nc = bass.Bass()
    t = {
        "xf": nc.dram_tensor("xf", [DIM, N], F32R, kind="ExternalInput"),
        "xt": nc.dram_tensor("xt", [DIM, TOK], F32R, kind="ExternalInput"),
        "wall": nc.dram_tensor("wall", [128, 8, 256], F32R,
                               kind="ExternalInput"),
        "g": nc.dram_tensor("g", [DIM], F32, kind="ExternalInput"),
        "onesr": nc.dram_tensor("onesr", [1, 128], F32R, kind="ExternalInput"),
        "br": nc.dram_tensor("br", [1, 256], F32R, kind="ExternalInput"),
        "ind2": nc.dram_tensor("ind2", [2, 128], F32R, kind="ExternalInput"),
        "oc2": nc.dram_tensor("oc2", [128, 2], F32R, kind="ExternalInput"),
        "bz": nc.dram_tensor("bz", [128, 128], F32R, kind="ExternalInput"),
        "yt": nc.dram_tensor("yt", [TOK, DIM], F32, kind="ExternalOutput"),
    }
    with tile.TileContext(nc) as tc:
        with ExitStack() as ctx:
            if niter > 1:
                pools = _make_pools(tc, ctx)
                for _ in range(niter):
                    _emit(nc, tc, ctx, t, pools)
            else:
                _emit(nc, tc, ctx, t)
    return nc


_NC_CACHE = {}


def _make_oc2():
    oc2 = np.zeros((128, 2), np.float32)
    oc2[0:64, 0] = 1.0
    oc2[64:128, 1] = 1.0
    return oc2


def _make_ind2():
    ind2 = np.zeros((2, 128), np.float32)
    ind2[0, 0:64] = SCALE
    ind2[1, 64:128] = SCALE
    return ind2


def _prep_inputs(x, w_qkv, w_out, b_out, g):
    xf = np.ascontiguousarray(x.reshape(DIM, N).astype(np.float32))
    w_q, w_k, w_v = (w_qkv[0:256], w_qkv[256:512], w_qkv[512:768])
    wqT = w_q.T.astype(np.float32)
    wkT = w_k.T.astype(np.float32)
    wvT = w_v.T.astype(np.float32)
    woT = w_out.T.astype(np.float32)
    wall = np.ascontiguousarray(np.stack(
        [wkT[0:128], wvT[0:128], wkT[128:256], wvT[128:256],
         wqT[0:128], wqT[128:256], woT[0:128], woT[128:256]], axis=1))
    common = {
        "xf": xf,
        "wall": wall,
        "g": np.ascontiguousarray(g.astype(np.float32)),
        "onesr": np.ones((1, 128), np.float32),
        "br": np.ascontiguousarray(b_out.astype(np.float32).reshape(1, 256)),
        "ind2": _make_ind2(),
        "oc2": _make_oc2(),
        "bz": np.zeros((128, 128), np.float32),
    }
    in_maps = []
    for c in range(NCORES):
        m = dict(common)
        m["xt"] = np.ascontiguousarray(xf[:, c * TOK:(c + 1) * TOK])
        in_maps.append(m)
    return in_maps


def kernel(x, w_qkv, w_out, b_out, g):
    _install_bir_fix()
    from concourse.bass_utils import run_bass_kernel_spmd

    if "nc" not in _NC_CACHE:
        _NC_CACHE["nc"] = build_nc()
    nc = _NC_CACHE["nc"]
    in_maps = _prep_inputs(np.asarray(x), np.asarray(w_qkv), np.asarray(w_out),
                           np.asarray(b_out), np.asarray(g))
    res = run_bass_kernel_spmd(nc, in_maps, core_ids=list(range(NCORES)))
    y = np.empty((DIM, N), np.float32)
    for c in range(NCORES):
        y[:, c * TOK:(c + 1) * TOK] = res.results[c]["yt"].T
    return y.reshape(1, DIM, 16, 16, 16)


if __name__ == "__main__":
    import reference as R
    inputs = {k: np.asarray(v) for k, v in R.setup_inputs().items()}
    ref = np.asarray(R.reference(**inputs))
    got = kernel(**inputs)
    err = np.abs(got - ref)
    print("rel_absmax:", err.max() / np.abs(ref).max())


# revision 17
# speedup vs baseline: 8.3277x; 8.3277x over previous
"""Trainium2 Bass kernel for nn_Attention3d_9483287790337.

Math: 1x1x1-conv QKV -> per-head (softmax_d q * scale) @ (softmax_n k) attention
over n=4096 tokens -> out proj -> channel LayerNorm.

Key numerical fact exploited: k's softmax is over the 4096 tokens, so k-tilde
entries are ~2.4e-4 and sim = q~^T k~ lies in [0, ~1.6e-4]. exp(sim) is linear
to ~8 significant digits (error ~sim^2/2 ~ 1e-8 relative), so
  attn = softmax(sim) = (1 + sim) / (n + SCALE)      [denominator is constant:
                                                      sum_j sim_ij = SCALE]
and the attention output factorizes through associativity:
  out = (vsum + B^T q~s) / (n*(n+SCALE)),  B[d',d] = sum_j ek~T[j,d'] vT[j,d]
This reduces the device work to ~n*d^2 instead of n^2*d.

Perf structure (vs the first working version, 126us -> ~36-40us measured
steady-state per iteration on HW; single-execution latency is lower):
  - All free-dim>=256 matmuls run as float32r (same bits, 1 cycle/row on the
    PE vs 4 for float32); bf16 for the small B-accumulation matmuls.
  - j-side sharded 8 ways: each core projects/exps only its own 512 tokens
    and the partial [B | rsk | vsum] (bf16, 67KB) is combined with one
    on-chip AllReduce (~7us), cutting PE/ACT/DVE work and HBM input traffic
    8x vs replicating the k/v side.
  - rsk (column sums of ekT) is folded into the B matmul as a ones column in
    the vT operand; vsum falls out of a 3rd accumulating ones-matmul; both
    come back partition-major via a PE transpose (no DRAM roundtrip).
  - k and v projections share one [wk|wv] moving operand so each stationary
    x block is loaded once; bias-add is a K=1 accumulating matmul; rstd via
    Ln/Exp on the hot ACT tables (no Sqrt table load, no DVE reciprocal).
  - 1/rsk is folded into the B block copy so q~s is a plain elementwise
    multiply; input/output DMAs are spread over the SP/ACT/Pool queues with
    fronts and tails on different queues so iterations pipeline.

Sharding: 8-way data parallel over tokens (512 tokens/core) on BOTH the
q side and the k/v side: each core computes partial B/rsk/vsum over its own
512 tokens, and one 133KB AllReduce (measured ~7us on-chip) combines them.
Weights replicated. This cuts the projection/B PE work, the ACT exp work,
and the HBM input traffic by 8x vs replicating the k/v side.
"""

import numpy as np
from contextlib import ExitStack

import concourse.bass as bass
import concourse.tile as tile
from concourse import mybir
import orjson

F32 = mybir.dt.float32
F32R = mybir.dt.float32r
BF16 = mybir.dt.bfloat16
AX = mybir.AxisListType
OP = mybir.AluOpType
AF = mybir.ActivationFunctionType

DIM = 256
HEADS = 4
DHEAD = 64
N = 4096           # tokens
TOK = 512          # tokens per core
NCORES = 8
SCALE = DHEAD ** -0.5
NORM = 1.0 / (N * (N + SCALE))
NJB = TOK // 128   # 4 local j-blocks per core (j-side sharded 8-way)
CHUNK = 2          # j-blocks per psum chunk in the kT/vT projections


def _r(ap):
    return ap.bitcast(F32R)


# --------------------------------------------------------------------------
# Workaround for this container's walrus build: its ISA encoding accepts at
# most ONE sync-wait per instruction, but tile.py emits several `on_wait`
# entries on one instruction. Split extras into single-wait NoOps on the same
# engine (engines execute their stream in order, so sequential waits are
# equivalent).
# --------------------------------------------------------------------------

_ENGINES = {"Pool", "Activation", "PE", "DVE", "SP"}
_SPLIT_OPCODE = "Drain"


def _split_multi_waits(bir_bytes: bytes) -> bytes:
    m = orjson.loads(bir_bytes)

    def walk(block):
        ins = block.get("instructions")
        if ins:
            out = []
            for inst in ins:
                si = inst.get("sync_info")
                waits = (si or {}).get("on_wait") or []
                if len(waits) > 1 and inst.get("engine") in _ENGINES:
                    for j, w in enumerate(waits[:-1]):
                        out.append({
                            "engine": inst["engine"],
                            "ins": [],
                            "outs": [],
                            "name": f"{inst.get('name', 'i')}_sw{j}",
                            "opcode": _SPLIT_OPCODE,
                            "sync_info": {"on_update": [], "on_wait": [w]},
                        })
                    si["on_wait"] = [waits[-1]]
                out.append(inst)
            block["instructions"] = out
        for sub in block.get("blocks") or []:
            walk(sub)

    for fn in m["functions"]:
        for b in fn["blocks"]:
            walk(b)
    return orjson.dumps(m)


_fix_installed = False


def _install_bir_fix():
    global _fix_installed
    if _fix_installed:
        return
    _fix_installed = True
    import concourse.bass_utils as bu
    import concourse.bass2jax as b2j

    orig = bu.compile_bir_kernel

    def patched(bir_json, tmpdir, neff_name="file.neff"):
        return orig(_split_multi_waits(bir_json), tmpdir, neff_name=neff_name)

    bu.compile_bir_kernel = patched
    b2j.compile_bir_kernel = patched


# --------------------------------------------------------------------------
# Device kernel
# --------------------------------------------------------------------------

def _make_pools(tc, ctx):
    # bufs=3: two iterations of cross-iteration pipelining slack when the
    # body is replicated for timing; harmless for a single execution.
    const = ctx.enter_context(tc.tile_pool(name="const", bufs=3))
    sb = ctx.enter_context(tc.tile_pool(name="sb", bufs=3))
    wk = ctx.enter_context(tc.tile_pool(name="wk", bufs=8))
    pp = ctx.enter_context(tc.tile_pool(name="pp", bufs=4, space="PSUM"))
    pacc = ctx.enter_context(tc.tile_pool(name="pacc", bufs=1, space="PSUM"))
    return const, sb, wk, pp, pacc


def _emit(nc, tc, ctx, t, pools=None):
    if pools is None:
        pools = _make_pools(tc, ctx)
    const, sb, wk, pp, pacc = pools

    # ---- constants / inputs to SBUF.
    # DMA queue plan: sync carries wall-half A then x half 0; gpsimd carries
    # wall-half B then x half 1; scalar carries the small constants. This gets
    # the first projection chunk running by ~7us instead of ~22us.
    # wall layout: [wkv0 (wk0|wv0), wkv1, wq0, wq1, wo0, wo1]
    wall = const.tile([128, 8, 256], F32R, name="wall", tag="wall")
    nc.sync.dma_start(out=wall[:, 0:4, :], in_=t["wall"][:, 0:4, :])
    nc.scalar.dma_start(out=wall[:, 4:8, :], in_=t["wall"][:, 4:8, :])
    wkv_sb = [wall[:, 0:2, :], wall[:, 2:4, :]]
    wq_sb = [wall[:, 4, :], wall[:, 5, :]]
    wo_sb = [wall[:, 6, :], wall[:, 7, :]]

    xt_sb = []
    for b in range(2):
        xtb = sb.tile([128, TOK], F32R, name=f"xt{b}", tag=f"xt{b}")
        eng = nc.sync if b == 0 else nc.scalar
        eng.dma_start(out=xtb, in_=t["xt"][b * 128:(b + 1) * 128, :])
        xt_sb.append(xtb)

    ones_col2 = const.tile([128, 2], F32R, name="ones_col2", tag="ones_col2")
    nc.scalar.dma_start(out=ones_col2, in_=t["oc2"][:, :])
    ind2_sb = const.tile([2, 128], F32R, name="ind2_sb", tag="ind2_sb")
    nc.scalar.dma_start(out=ind2_sb, in_=t["ind2"][:, :])

    g_bc = const.tile([128, 256], F32, name="g_bc", tag="g_bc")
    ap = t["g"][:]
    bcast = bass.AP(tensor=ap.tensor, offset=ap.offset,
                    ap=[[0, 128]] + list(ap.ap))
    nc.scalar.dma_start(out=g_bc, in_=bcast)
    onesr_sb = const.tile([1, 128], F32R, name="onesr_sb", tag="onesr_sb")
    nc.scalar.dma_start(out=onesr_sb, in_=t["onesr"][:, :])
    br_sb = const.tile([1, 256], F32R, name="br_sb", tag="br_sb")
    nc.scalar.dma_start(out=br_sb, in_=t["br"][:, :])

    ones_bf = const.tile([128, 1], BF16, name="ones_bf", tag="ones_bf")
    nc.vector.memset(ones_bf, 1.0)
    ones_f1 = const.tile([1, 1], F32, name="ones_f1", tag="ones_f1")
    nc.vector.memset(ones_f1, 1.0)
    eps_t = const.tile([128, 1], F32, name="eps_t", tag="eps_t")
    nc.vector.memset(eps_t, 1e-5)

    # ---- token-major ekT / vT stores (bf16). vT layout per j-block:
    # [one, v_cb0 (128), one, v_cb1 (128)] so each cb's B matmul rhs is the
    # contiguous 129 columns [ones | v_half], yielding rsk in pB column 0.
    ekT = sb.tile([128, NJB, 256], BF16, name="ekT", tag="ekT")
    vt = sb.tile([128, NJB, 258], BF16, name="vt", tag="vt")
    nc.vector.memset(vt[:, :, 0:1], 1.0)
    nc.vector.memset(vt[:, :, 129:130], 1.0)

    # ---- q path: q = w_q @ xt ; eq = exp(q); per-head rowsums over d
    eq_sb, rcp2_sb = [], []
    for cb in range(2):
        pq = pp.tile([128, TOK], F32, name=f"pq{cb}", tag="p")
        for inb in range(2):
            nc.tensor.matmul(pq, wq_sb[inb][:, cb * 128:(cb + 1) * 128],
                             xt_sb[inb], start=(inb == 0), stop=(inb == 1))
        eq = sb.tile([128, TOK], F32R, name=f"eq{cb}", tag=f"eq{cb}")
        nc.scalar.activation(out=eq, in_=pq, func=AF.Exp)
        eq_sb.append(eq)
        # per-head-half column sums over the 128 partitions -> [2, TOK]
        prs = pp.tile([2, TOK], F32, name=f"prs{cb}", tag="p")
        nc.tensor.matmul(prs, ones_col2, eq, start=True, stop=True)
        # 1/x as exp(-ln(x)) on ACT (custom-DVE recip ops don't lower in
        # this walrus build; DVE iterative divide is slow on 2 partitions)
        lnp = const.tile([2, TOK], F32, name=f"lnp{cb}", tag=f"lnp{cb}")
        nc.scalar.activation(out=lnp, in_=prs, func=AF.Ln)
        rcp2 = const.tile([2, TOK], F32R, name=f"rcp2{cb}", tag=f"rcp2{cb}")
        nc.scalar.activation(out=rcp2, in_=lnp, func=AF.Exp, scale=-1.0)
        rcp2_sb.append(rcp2)

    # ---- fused kT/vT projections + B/rsk/vsum accumulation
    # pB[cb][d', 0] = rsk over cb's channels; pB[cb][d', 1+d] = B[d', d].
    # pvs[0, 1+129*cb+d] = vsum[d] for cb's channels.
    pB = [pacc.tile([128, 132], F32, name=f"pB{cb}", tag=f"accB{cb}")
          for cb in range(2)]
    B_bd = [sb.tile([128, 128], F32R, name=f"Bbd{cb}", tag=f"Bbd{cb}")
            for cb in range(2)]
    for cb in range(2):
        nc.scalar.dma_start(out=B_bd[cb], in_=t["bz"][:, :])
    pvs = pacc.tile([1, 258], F32, name="pvs", tag="accv")
    for jb in range(NJB):
        pkv = pp.tile([128, 2, 256], F32, name=f"pkv{jb}", tag="p")
        for inb in range(2):
            nc.tensor.matmul(pkv, xt_sb[inb][:, jb * 128:(jb + 1) * 128],
                             wkv_sb[inb], start=(inb == 0), stop=(inb == 1))
        nc.scalar.activation(out=ekT[:, jb, :], in_=pkv[:, 0, :], func=AF.Exp)
        # copy v -> vt halves (cast to bf16) in one strided ACT op; the
        # out AP jumps 129 cols between halves to skip the ones columns
        vslice = bass.AP(tensor=vt.tensor, offset=vt.offset + jb * 258 + 1,
                         ap=[list(vt.ap[0]), [129, 2], [1, 128]])
        nc.scalar.copy(out=vslice, in_=pkv[:, 1, :])
        for cb in range(2):
            nc.tensor.matmul(pB[cb][:, 0:129],
                             ekT[:, jb, cb * 128:(cb + 1) * 128],
                             vt[:, jb, cb * 129:cb * 129 + 129],
                             start=(jb == 0), stop=(jb == NJB - 1))
        nc.tensor.matmul(pvs, ones_bf, vt[:, jb, :],
                         start=(jb == 0), stop=(jb == NJB - 1))

    # ---- AllReduce the per-core partial [B0|rsk0 , B1|rsk1 , vsum] across
    # the 8 cores: pack to DRAM, CC, read the reduced copy back.
    Bp_sb = sb.tile([128, 258], BF16, name="Bp_sb", tag="Bp_sb")
    for cb in range(2):
        nc.vector.tensor_copy(Bp_sb[:, 129 * cb:129 * cb + 129],
                              pB[cb][:, 0:129])
    pvs_row = const.tile([1, 258], BF16, name="pvs_row", tag="pvs_row")
    nc.vector.tensor_copy(pvs_row, pvs)
    nc.sync.dma_start(out=t["cc_in"][0:128, :], in_=Bp_sb)
    nc.scalar.dma_start(out=t["cc_in"][128:129, :], in_=pvs_row)
    nc.gpsimd.collective_compute(
        "AllReduce", OP.add, replica_groups=[list(range(NCORES))],
        ins=[t["cc_in"][:]], outs=[t["cc_out"][:]])
    rBs = sb.tile([128, 258], BF16, name="rBs", tag="Bp_sb")
    nc.sync.dma_start(out=rBs, in_=t["cc_out"][0:128, :])
    pvs_row2 = const.tile([1, 258], BF16, name="pvs_row2", tag="pvs_row2")
    nc.scalar.dma_start(out=pvs_row2, in_=t["cc_out"][128:129, :])

    # ---- rk (1/rsk), B blocks, vsum to partition-major
    rk_p = const.tile([128, 2], F32, name="rk_p", tag="rk_p")
    for cb in range(2):
        nc.vector.reciprocal(rk_p[:, cb:cb + 1], rBs[:, 129 * cb:129 * cb + 1])
        for hp in range(2):
            r0 = 64 * hp
            # scale B rows by rk[d'] here so qs needs no rk factor
            nc.vector.tensor_scalar(out=B_bd[cb][r0:r0 + 64, r0:r0 + 64],
                                    in0=rBs[r0:r0 + 64, 129 * cb + 1 + r0:
                                            129 * cb + 1 + r0 + 64],
                                    scalar1=rk_p[r0:r0 + 64, cb:cb + 1],
                                    scalar2=None, op0=OP.mult)
    pvf = const.tile([1, 258], F32, name="pvf", tag="pvf")
    nc.vector.tensor_copy(pvf, pvs_row2)
    pvT = pacc.tile([128, 2], F32, name="pvT", tag="accv2")
    for cb in range(2):
        # [1,128] free-major -> [128,1] partition-major via PE transpose
        nc.tensor.transpose(pvT[:, cb:cb + 1],
                            pvf[0:1, 1 + 129 * cb:129 + 129 * cb],
                            ones_f1)

    # ---- q~s = eq * rk[d'] * (SCALE * rcp2[half(d'), i]) ; numer ; out_all
    out_all = []
    for cb in range(2):
        qsc = pp.tile([128, TOK], F32, name=f"qsc{cb}", tag="p")
        nc.tensor.matmul(qsc, ind2_sb, rcp2_sb[cb], start=True,
                         stop=True)
        qs = sb.tile([128, TOK], F32R, name=f"qs{cb}", tag=f"qs{cb}")
        pnum = pp.tile([128, TOK], F32, name=f"pnum{cb}", tag="p")
        oa = sb.tile([128, TOK], F32R, name=f"oall{cb}", tag=f"oall{cb}")
        for th in range(2):
            sl = slice(th * 256, (th + 1) * 256)
            nc.vector.tensor_mul(out=qs[:, sl],
                                 in0=eq_sb[cb].bitcast(F32)[:, sl],
                                 in1=qsc[:, sl])
            nc.tensor.matmul(pnum[:, sl], B_bd[cb], qs[:, sl], start=True,
                             stop=True)
            nc.vector.tensor_scalar(out=oa[:, sl], in0=pnum[:, sl],
                                    scalar1=pvT[:, cb:cb + 1], scalar2=NORM,
                                    op0=OP.add, op1=OP.mult)
        out_all.append(oa)

    # ---- y = w_out @ out + b_out (as yT [t, o]), then channel LayerNorm
    yo4 = wk.tile([128, 4, 256], F32, name="yo4", tag="w_yo4", bufs=2)
    for tb in range(TOK // 128):
        py = pp.tile([128, 256], F32, name=f"py{tb}", tag="p")
        for cb in range(2):
            nc.tensor.matmul(py, out_all[cb][:, tb * 128:(tb + 1) * 128],
                             wo_sb[cb], start=(cb == 0), stop=False)
        # bias add as a K=1 accumulating matmul: py += ones^T @ b_row
        nc.tensor.matmul(py, onesr_sb, br_sb, start=False, stop=True)
        stats = wk.tile([128, 6], F32, name=f"st{tb}", tag="w_small")
        nc.vector.bn_stats(out=stats, in_=py)
        mv = wk.tile([128, 2], F32, name=f"mv{tb}", tag="w_small")
        nc.vector.bn_aggr(out=mv, in_=stats)
        lnv = wk.tile([128, 1], F32, name=f"lv{tb}", tag="w_small")
        nc.scalar.activation(out=lnv, in_=mv[:, 1:2], func=AF.Ln, bias=eps_t)
        rstd = wk.tile([128, 1], F32, name=f"rs{tb}", tag="w_small")
        nc.scalar.activation(out=rstd, in_=lnv, func=AF.Exp, scale=-0.5)
        yn = wk.tile([128, 256], F32, name=f"yn{tb}", tag="w_yb")
        nc.vector.tensor_scalar(out=yn, in0=py, scalar1=mv[:, 0:1],
                                scalar2=rstd, op0=OP.subtract, op1=OP.mult)
        nc.gpsimd.tensor_mul(out=yo4[:, tb, :], in0=yn, in1=g_bc)
        if tb % 2 == 1:
            eng = nc.sync if tb == 1 else nc.scalar
            # one DMA covers two 128-token blocks: DRAM rows (tb*128+p) for
            # tb in {tb-1, tb}
            dst = bass.AP(tensor=t["yt"], offset=(tb - 1) * 128 * 256,
                          ap=[[256, 128], [128 * 256, 2], [1, 256]])
            eng.dma_start(out=dst, in_=yo4[:, tb - 1:tb + 1, :])


def build_nc(niter=1):
    nc = bass.Bass(num_devices=NCORES)
    t = {
        "xt": nc.dram_tensor("xt", [DIM, TOK], F32R, kind="ExternalInput"),
        "cc_in": nc.dram_tensor("cc_in", [129, 258], BF16),
        "cc_out": nc.dram_tensor("cc_out", [129, 258], BF16,
                                 addr_space="Shared"),
        "wall": nc.dram_tensor("wall", [128, 8, 256], F32R,
                               kind="ExternalInput"),
        "g": nc.dram_tensor("g", [DIM], F32, kind="ExternalInput"),
        "onesr": nc.dram_tensor("onesr", [1, 128], F32R, kind="ExternalInput"),
        "br": nc.dram_tensor("br", [1, 256], F32R, kind="ExternalInput"),
        "ind2": nc.dram_tensor("ind2", [2, 128], F32R, kind="ExternalInput"),
        "oc2": nc.dram_tensor("oc2", [128, 2], F32R, kind="ExternalInput"),
        "bz": nc.dram_tensor("bz", [128, 128], F32R, kind="ExternalInput"),
        "yt": nc.dram_tensor("yt", [TOK, DIM], F32, kind="ExternalOutput"),
    }
    with tile.TileContext(nc) as tc:
        with ExitStack() as ctx:
            if niter > 1:
                pools = _make_pools(tc, ctx)
                for _ in range(niter):
                    _emit(nc, tc, ctx, t, pools)
            else:
                _emit(nc, tc, ctx, t)
    return nc


_NC_CACHE = {}


def _make_oc2():
    oc2 = np.zeros((128, 2), np.float32)
    oc2[0:64, 0] = 1.0
    oc2[64:128, 1] = 1.0
    return oc2


def _make_ind2():
    ind2 = np.zeros((2, 128), np.float32)
    ind2[0, 0:64] = SCALE
    ind2[1, 64:128] = SCALE
    return ind2


def _prep_inputs(x, w_qkv, w_out, b_out, g):
    xf = np.ascontiguousarray(x.reshape(DIM, N).astype(np.float32))
    w_q, w_k, w_v = (w_qkv[0:256], w_qkv[256:512], w_qkv[512:768])
    wqT = w_q.T.astype(np.float32)
    wkT = w_k.T.astype(np.float32)
    wvT = w_v.T.astype(np.float32)
    woT = w_out.T.astype(np.float32)
    wall = np.ascontiguousarray(np.stack(
        [wkT[0:128], wvT[0:128], wkT[128:256], wvT[128:256],
         wqT[0:128], wqT[128:256], woT[0:128], woT[128:256]], axis=1))
    common = {
        "wall": wall,
        "g": np.ascontiguousarray(g.astype(np.float32)),
        "onesr": np.ones((1, 128), np.float32),
        "br": np.ascontiguousarray(b_out.astype(np.float32).reshape(1, 256)),
        "ind2": _make_ind2(),
        "oc2": _make_oc2(),
        "bz": np.zeros((128, 128), np.float32),
    }
    in_maps = []
    for c in range(NCORES):
        m = dict(common)
        m["xt"] = np.ascontiguousarray(xf[:, c * TOK:(c + 1) * TOK])
        in_maps.append(m)
    return in_maps


def kernel(x, w_qkv, w_out, b_out, g):
    _install_bir_fix()
    from concourse.bass_utils import run_bass_kernel_spmd

    if "nc" not in _NC_CACHE:
        _NC_CACHE["nc"] = build_nc()
    nc = _NC_CACHE["nc"]
    in_maps = _prep_inputs(np.asarray(x), np.asarray(w_qkv), np.asarray(w_out),
                           np.asarray(b_out), np.asarray(g))
    res = run_bass_kernel_spmd(nc, in_maps, core_ids=list(range(NCORES)))
    y = np.empty((DIM, N), np.float32)
    for c in range(NCORES):
        y[:, c * TOK:(c + 1) * TOK] = res.results[c]["yt"].T
    return y.reshape(1, DIM, 16, 16, 16)


if __name__ == "__main__":
    import reference as R
    inputs = {k: np.asarray(v) for k, v in R.setup_inputs().items()}
    ref = np.asarray(R.reference(**inputs))
    got = kernel(**inputs)
    err = np.abs(got - ref)
    print("rel_absmax:", err.max() / np.abs(ref).max())


# revision 18
# speedup vs baseline: 12.3516x; 1.4832x over previous
"""Trainium2 Bass kernel for nn_Attention3d_9483287790337.

Math: 1x1x1-conv QKV -> per-head (softmax_d q * scale) @ (softmax_n k) attention
over n=4096 tokens -> out proj -> channel LayerNorm.

Key numerical fact exploited: k's softmax is over the 4096 tokens, so k-tilde
entries are ~2.4e-4 and sim = q~^T k~ lies in [0, ~1.6e-4]. exp(sim) is linear
to ~8 significant digits (error ~sim^2/2 ~ 1e-8 relative), so
  attn = softmax(sim) = (1 + sim) / (n + SCALE)      [denominator is constant:
                                                      sum_j sim_ij = SCALE]
and the attention output factorizes through associativity:
  out = (vsum + B^T q~s) / (n*(n+SCALE)),  B[d',d] = sum_j ek~T[j,d'] vT[j,d]
This reduces the device work to ~n*d^2 instead of n^2*d.

Perf structure (vs the first working version, 126us -> ~36-40us measured
steady-state per iteration on HW; single-execution latency is lower):
  - All free-dim>=256 matmuls run as float32r (same bits, 1 cycle/row on the
    PE vs 4 for float32); bf16 for the small B-accumulation matmuls.
  - j-side sharded 8 ways: each core projects/exps only its own 512 tokens
    and the partial [B | rsk | vsum] (bf16, 67KB) is combined with one
    on-chip AllReduce (~7us), cutting PE/ACT/DVE work and HBM input traffic
    8x vs replicating the k/v side.
  - rsk (column sums of ekT) is folded into the B matmul as a ones column in
    the vT operand; vsum falls out of a 3rd accumulating ones-matmul; both
    come back partition-major via a PE transpose (no DRAM roundtrip).
  - k and v projections share one [wk|wv] moving operand so each stationary
    x block is loaded once; bias-add is a K=1 accumulating matmul; rstd via
    Ln/Exp on the hot ACT tables (no Sqrt table load, no DVE reciprocal).
  - 1/rsk is folded into the B block copy so q~s is a plain elementwise
    multiply; input/output DMAs are spread over the SP/ACT/Pool queues with
    fronts and tails on different queues so iterations pipeline.

Sharding: 8-way data parallel over tokens (512 tokens/core) on BOTH the
q side and the k/v side: each core computes partial B/rsk/vsum over its own
512 tokens, and one 133KB AllReduce (measured ~7us on-chip) combines them.
Weights replicated. This cuts the projection/B PE work, the ACT exp work,
and the HBM input traffic by 8x vs replicating the k/v side.
"""

import numpy as np
from contextlib import ExitStack

import concourse.bass as bass
import concourse.tile as tile
from concourse import mybir
import orjson

F32 = mybir.dt.float32
F32R = mybir.dt.float32r
BF16 = mybir.dt.bfloat16
AX = mybir.AxisListType
OP = mybir.AluOpType
AF = mybir.ActivationFunctionType

DIM = 256
HEADS = 4
DHEAD = 64
N = 4096           # tokens
TOK = 512          # tokens per core
NCORES = 8
SCALE = DHEAD ** -0.5
NORM = 1.0 / (N * (N + SCALE))
NJB = TOK // 128   # 4 local j-blocks per core (j-side sharded 8-way)
CHUNK = 2          # j-blocks per psum chunk in the kT/vT projections


def _r(ap):
    return ap.bitcast(F32R)


# --------------------------------------------------------------------------
# Workaround for this container's walrus build: its ISA encoding accepts at
# most ONE sync-wait per instruction, but tile.py emits several `on_wait`
# entries on one instruction. Split extras into single-wait NoOps on the same
# engine (engines execute their stream in order, so sequential waits are
# equivalent).
# --------------------------------------------------------------------------

_ENGINES = {"Pool", "Activation", "PE", "DVE", "SP"}
_SPLIT_OPCODE = "Drain"


def _split_multi_waits(bir_bytes: bytes) -> bytes:
    m = orjson.loads(bir_bytes)

    def walk(block):
        ins = block.get("instructions")
        if ins:
            out = []
            for inst in ins:
                si = inst.get("sync_info")
                waits = (si or {}).get("on_wait") or []
                if len(waits) > 1 and inst.get("engine") in _ENGINES:
                    for j, w in enumerate(waits[:-1]):
                        out.append({
                            "engine": inst["engine"],
                            "ins": [],
                            "outs": [],
                            "name": f"{inst.get('name', 'i')}_sw{j}",
                            "opcode": _SPLIT_OPCODE,
                            "sync_info": {"on_update": [], "on_wait": [w]},
                        })
                    si["on_wait"] = [waits[-1]]
                out.append(inst)
            block["instructions"] = out
        for sub in block.get("blocks") or []:
            walk(sub)

    for fn in m["functions"]:
        for b in fn["blocks"]:
            walk(b)
    return orjson.dumps(m)


_fix_installed = False


def _install_bir_fix():
    global _fix_installed
    if _fix_installed:
        return
    _fix_installed = True
    import concourse.bass_utils as bu
    import concourse.bass2jax as b2j

    orig = bu.compile_bir_kernel

    def patched(bir_json, tmpdir, neff_name="file.neff"):
        return orig(_split_multi_waits(bir_json), tmpdir, neff_name=neff_name)

    bu.compile_bir_kernel = patched
    b2j.compile_bir_kernel = patched


# --------------------------------------------------------------------------
# Device kernel
# --------------------------------------------------------------------------

def _make_pools(tc, ctx):
    # bufs=3: two iterations of cross-iteration pipelining slack when the
    # body is replicated for timing; harmless for a single execution.
    const = ctx.enter_context(tc.tile_pool(name="const", bufs=3))
    sb = ctx.enter_context(tc.tile_pool(name="sb", bufs=3))
    wk = ctx.enter_context(tc.tile_pool(name="wk", bufs=8))
    # front-phase ring ("p", bufs=2) and tail-phase ring ("pt", bufs=2) are
    # separate so the next iteration's projections never queue behind this
    # iteration's LayerNorm reads of the py tiles
    pp = ctx.enter_context(tc.tile_pool(name="pp", bufs=2, space="PSUM"))
    pacc = ctx.enter_context(tc.tile_pool(name="pacc", bufs=1, space="PSUM"))
    return const, sb, wk, pp, pacc


def _emit(nc, tc, ctx, t, pools=None):
    if pools is None:
        pools = _make_pools(tc, ctx)
    const, sb, wk, pp, pacc = pools

    # ---- constants / inputs to SBUF.
    # DMA queue plan: sync carries wall-half A then x half 0; gpsimd carries
    # wall-half B then x half 1; scalar carries the small constants. This gets
    # the first projection chunk running by ~7us instead of ~22us.
    # wall layout: [wkv0 (wk0|wv0), wkv1, wq0, wq1, wo0, wo1]
    wall = const.tile([128, 8, 256], F32R, name="wall", tag="wall")
    nc.sync.dma_start(out=wall[:, 0:4, :], in_=t["wall"][:, 0:4, :])
    nc.scalar.dma_start(out=wall[:, 4:8, :], in_=t["wall"][:, 4:8, :])
    wkv_sb = [wall[:, 0:2, :], wall[:, 2:4, :]]
    wq_sb = [wall[:, 4, :], wall[:, 5, :]]
    wo_sb = [wall[:, 6, :], wall[:, 7, :]]

    xt_sb = []
    for b in range(2):
        xtb = sb.tile([128, TOK], F32R, name=f"xt{b}", tag=f"xt{b}")
        eng = nc.sync if b == 0 else nc.scalar
        eng.dma_start(out=xtb, in_=t["xt"][b * 128:(b + 1) * 128, :])
        xt_sb.append(xtb)

    ones_col2 = const.tile([128, 2], F32R, name="ones_col2", tag="ones_col2")
    nc.scalar.dma_start(out=ones_col2, in_=t["oc2"][:, :])
    ind2_sb = const.tile([2, 128], F32R, name="ind2_sb", tag="ind2_sb")
    nc.scalar.dma_start(out=ind2_sb, in_=t["ind2"][:, :])

    g_bc = const.tile([128, 256], F32, name="g_bc", tag="g_bc")
    ap = t["g"][:]
    bcast = bass.AP(tensor=ap.tensor, offset=ap.offset,
                    ap=[[0, 128]] + list(ap.ap))
    nc.scalar.dma_start(out=g_bc, in_=bcast)
    onesr_sb = const.tile([1, 128], F32R, name="onesr_sb", tag="onesr_sb")
    nc.scalar.dma_start(out=onesr_sb, in_=t["onesr"][:, :])
    br_sb = const.tile([1, 256], F32R, name="br_sb", tag="br_sb")
    nc.scalar.dma_start(out=br_sb, in_=t["br"][:, :])

    ones_bf = const.tile([128, 1], BF16, name="ones_bf", tag="ones_bf")
    nc.vector.memset(ones_bf, 1.0)
    ones_f1 = const.tile([1, 1], F32, name="ones_f1", tag="ones_f1")
    nc.vector.memset(ones_f1, 1.0)
    eps_t = const.tile([128, 1], F32, name="eps_t", tag="eps_t")
    nc.vector.memset(eps_t, 1e-5)

    # ---- token-major ekT / vT stores (bf16). vT layout per j-block:
    # [one, v_cb0 (128), one, v_cb1 (128)] so each cb's B matmul rhs is the
    # contiguous 129 columns [ones | v_half], yielding rsk in pB column 0.
    ekT = sb.tile([128, NJB, 256], BF16, name="ekT", tag="ekT")
    vt = sb.tile([128, NJB, 258], BF16, name="vt", tag="vt")
    nc.vector.memset(vt[:, :, 0:1], 1.0)
    nc.vector.memset(vt[:, :, 129:130], 1.0)

    # ---- q path: q = w_q @ xt ; eq = exp(q); per-head rowsums over d
    eq_sb, rcp2_sb = [], []
    for cb in range(2):
        pq = pp.tile([128, TOK], F32, name=f"pq{cb}", tag="p")
        for inb in range(2):
            nc.tensor.matmul(pq, wq_sb[inb][:, cb * 128:(cb + 1) * 128],
                             xt_sb[inb], start=(inb == 0), stop=(inb == 1))
        eq = sb.tile([128, TOK], F32R, name=f"eq{cb}", tag=f"eq{cb}")
        nc.scalar.activation(out=eq, in_=pq, func=AF.Exp)
        eq_sb.append(eq)
        # per-head-half column sums over the 128 partitions -> [2, TOK]
        prs = pp.tile([2, TOK], F32, name=f"prs{cb}", tag="p")
        nc.tensor.matmul(prs, ones_col2, eq, start=True, stop=True)
        # 1/x as exp(-ln(x)) on ACT (custom-DVE recip ops don't lower in
        # this walrus build; DVE iterative divide is slow on 2 partitions)
        lnp = const.tile([2, TOK], F32, name=f"lnp{cb}", tag=f"lnp{cb}")
        nc.scalar.activation(out=lnp, in_=prs, func=AF.Ln)
        rcp2 = const.tile([2, TOK], F32R, name=f"rcp2{cb}", tag=f"rcp2{cb}")
        nc.scalar.activation(out=rcp2, in_=lnp, func=AF.Exp, scale=-1.0)
        rcp2_sb.append(rcp2)

    # ---- fused kT/vT projections + B/rsk/vsum accumulation
    # pB[cb][d', 0] = rsk over cb's channels; pB[cb][d', 1+d] = B[d', d].
    # pvs[0, 1+129*cb+d] = vsum[d] for cb's channels.
    pB = [pacc.tile([128, 132], F32, name=f"pB{cb}", tag=f"accB{cb}")
          for cb in range(2)]
    B_bd = [sb.tile([128, 128], F32R, name=f"Bbd{cb}", tag=f"Bbd{cb}")
            for cb in range(2)]
    for cb in range(2):
        nc.scalar.dma_start(out=B_bd[cb], in_=t["bz"][:, :])
    pvs = pacc.tile([1, 258], F32, name="pvs", tag="accv")
    for jb in range(NJB):
        pkv = pp.tile([128, 2, 256], F32, name=f"pkv{jb}", tag="p")
        for inb in range(2):
            nc.tensor.matmul(pkv, xt_sb[inb][:, jb * 128:(jb + 1) * 128],
                             wkv_sb[inb], start=(inb == 0), stop=(inb == 1))
        nc.scalar.activation(out=ekT[:, jb, :], in_=pkv[:, 0, :], func=AF.Exp)
        # copy v -> vt halves (cast to bf16) in one strided ACT op; the
        # out AP jumps 129 cols between halves to skip the ones columns
        vslice = bass.AP(tensor=vt.tensor, offset=vt.offset + jb * 258 + 1,
                         ap=[list(vt.ap[0]), [129, 2], [1, 128]])
        nc.scalar.copy(out=vslice, in_=pkv[:, 1, :])
        for cb in range(2):
            nc.tensor.matmul(pB[cb][:, 0:129],
                             ekT[:, jb, cb * 128:(cb + 1) * 128],
                             vt[:, jb, cb * 129:cb * 129 + 129],
                             start=(jb == 0), stop=(jb == NJB - 1))
        nc.tensor.matmul(pvs, ones_bf, vt[:, jb, :],
                         start=(jb == 0), stop=(jb == NJB - 1))

    # ---- AllReduce the per-core partial [B0|rsk0 , B1|rsk1 , vsum] across
    # the 8 cores: pack to DRAM, CC, read the reduced copy back.
    Bp_sb = sb.tile([128, 258], BF16, name="Bp_sb", tag="Bp_sb")
    for cb in range(2):
        nc.vector.tensor_copy(Bp_sb[:, 129 * cb:129 * cb + 129],
                              pB[cb][:, 0:129])
    pvs_row = const.tile([1, 258], BF16, name="pvs_row", tag="pvs_row")
    nc.vector.tensor_copy(pvs_row, pvs)
    nc.sync.dma_start(out=t["cc_in"][0:128, :], in_=Bp_sb)
    nc.scalar.dma_start(out=t["cc_in"][128:129, :], in_=pvs_row)
    nc.gpsimd.collective_compute(
        "AllReduce", OP.add, replica_groups=[list(range(NCORES))],
        ins=[t["cc_in"][:]], outs=[t["cc_out"][:]])
    rBs = sb.tile([128, 258], BF16, name="rBs", tag="Bp_sb")
    nc.sync.dma_start(out=rBs, in_=t["cc_out"][0:128, :])
    pvs_row2 = const.tile([1, 258], BF16, name="pvs_row2", tag="pvs_row2")
    nc.scalar.dma_start(out=pvs_row2, in_=t["cc_out"][128:129, :])

    # ---- rk (1/rsk), B blocks, vsum to partition-major
    rk_p = const.tile([128, 2], F32, name="rk_p", tag="rk_p")
    for cb in range(2):
        nc.vector.reciprocal(rk_p[:, cb:cb + 1], rBs[:, 129 * cb:129 * cb + 1])
        for hp in range(2):
            r0 = 64 * hp
            # scale B rows by rk[d'] here so qs needs no rk factor
            nc.vector.tensor_scalar(out=B_bd[cb][r0:r0 + 64, r0:r0 + 64],
                                    in0=rBs[r0:r0 + 64, 129 * cb + 1 + r0:
                                            129 * cb + 1 + r0 + 64],
                                    scalar1=rk_p[r0:r0 + 64, cb:cb + 1],
                                    scalar2=None, op0=OP.mult)
    pvf = const.tile([1, 258], F32, name="pvf", tag="pvf")
    nc.vector.tensor_copy(pvf, pvs_row2)
    pvT = pacc.tile([128, 2], F32, name="pvT", tag="accv2")
    for cb in range(2):
        # [1,128] free-major -> [128,1] partition-major via PE transpose
        nc.tensor.transpose(pvT[:, cb:cb + 1],
                            pvf[0:1, 1 + 129 * cb:129 + 129 * cb],
                            ones_f1)

    # ---- q~s = eq * rk[d'] * (SCALE * rcp2[half(d'), i]) ; numer ; out_all
    out_all = []
    for cb in range(2):
        qsc = pp.tile([128, TOK], F32, name=f"qsc{cb}", tag="pt")
        nc.tensor.matmul(qsc, ind2_sb, rcp2_sb[cb], start=True,
                         stop=True)
        qs = sb.tile([128, TOK], F32R, name=f"qs{cb}", tag=f"qs{cb}")
        pnum = pp.tile([128, TOK], F32, name=f"pnum{cb}", tag="pt")
        oa = sb.tile([128, TOK], F32R, name=f"oall{cb}", tag=f"oall{cb}")
        for th in range(2):
            sl = slice(th * 256, (th + 1) * 256)
            nc.vector.tensor_mul(out=qs[:, sl],
                                 in0=eq_sb[cb].bitcast(F32)[:, sl],
                                 in1=qsc[:, sl])
            nc.tensor.matmul(pnum[:, sl], B_bd[cb], qs[:, sl], start=True,
                             stop=True)
            nc.vector.tensor_scalar(out=oa[:, sl], in0=pnum[:, sl],
                                    scalar1=pvT[:, cb:cb + 1], scalar2=NORM,
                                    op0=OP.add, op1=OP.mult)
        out_all.append(oa)

    # ---- y = w_out @ out + b_out (as yT [t, o]), then channel LayerNorm
    yo4 = wk.tile([128, 4, 256], F32, name="yo4", tag="w_yo4", bufs=2)
    for tb in range(TOK // 128):
        py = pp.tile([128, 256], F32, name=f"py{tb}", tag="pt")
        for cb in range(2):
            nc.tensor.matmul(py, out_all[cb][:, tb * 128:(tb + 1) * 128],
                             wo_sb[cb], start=(cb == 0), stop=False)
        # bias add as a K=1 accumulating matmul: py += ones^T @ b_row
        nc.tensor.matmul(py, onesr_sb, br_sb, start=False, stop=True)
        stats = wk.tile([128, 6], F32, name=f"st{tb}", tag="w_small")
        nc.vector.bn_stats(out=stats, in_=py)
        mv = wk.tile([128, 2], F32, name=f"mv{tb}", tag="w_small")
        nc.vector.bn_aggr(out=mv, in_=stats)
        lnv = wk.tile([128, 1], F32, name=f"lv{tb}", tag="w_small")
        nc.scalar.activation(out=lnv, in_=mv[:, 1:2], func=AF.Ln, bias=eps_t)
        rstd = wk.tile([128, 1], F32, name=f"rs{tb}", tag="w_small")
        nc.scalar.activation(out=rstd, in_=lnv, func=AF.Exp, scale=-0.5)
        yn = wk.tile([128, 256], F32, name=f"yn{tb}", tag="w_yb")
        nc.vector.tensor_scalar(out=yn, in0=py, scalar1=mv[:, 0:1],
                                scalar2=rstd, op0=OP.subtract, op1=OP.mult)
        nc.gpsimd.tensor_mul(out=yo4[:, tb, :], in0=yn, in1=g_bc)
        if tb % 2 == 1:
            eng = nc.sync if tb == 1 else nc.scalar
            # one DMA covers two 128-token blocks: DRAM rows (tb*128+p) for
            # tb in {tb-1, tb}
            dst = bass.AP(tensor=t["yt"], offset=(tb - 1) * 128 * 256,
                          ap=[[256, 128], [128 * 256, 2], [1, 256]])
            eng.dma_start(out=dst, in_=yo4[:, tb - 1:tb + 1, :])


def build_nc(niter=1):
    nc = bass.Bass(num_devices=NCORES)
    t = {
        "xt": nc.dram_tensor("xt", [DIM, TOK], F32R, kind="ExternalInput"),
        "cc_in": nc.dram_tensor("cc_in", [129, 258], BF16),
        "cc_out": nc.dram_tensor("cc_out", [129, 258], BF16,
                                 addr_space="Shared"),
        "wall": nc.dram_tensor("wall", [128, 8, 256], F32R,
                               kind="ExternalInput"),
        "g": nc.dram_tensor("g", [DIM], F32, kind="ExternalInput"),
        "onesr": nc.dram_tensor("onesr", [1, 128], F32R, kind="ExternalInput"),
        "br": nc.dram_tensor("br", [1, 256], F32R, kind="ExternalInput"),
        "ind2": nc.dram_tensor("ind2", [2, 128], F32R, kind="ExternalInput"),
        "oc2": nc.dram_tensor("oc2", [128, 2], F32R, kind="ExternalInput"),
        "bz": nc.dram_tensor("bz", [128, 128], F32R, kind="ExternalInput"),
        "yt": nc.dram_tensor("yt", [TOK, DIM], F32, kind="ExternalOutput"),
    }
    with tile.TileContext(nc) as tc:
        with ExitStack() as ctx:
            if niter > 1:
                pools = _make_pools(tc, ctx)
                for _ in range(niter):
                    _emit(nc, tc, ctx, t, pools)
            else:
                _emit(nc, tc, ctx, t)
    return nc


_NC_CACHE = {}


def _make_oc2():
    oc2 = np.zeros((128, 2), np.float32)
    oc2[0:64, 0] = 1.0
    oc2[64:128, 1] = 1.0
    return oc2


def _make_ind2():
    ind2 = np.zeros((2, 128), np.float32)
    ind2[0, 0:64] = SCALE
    ind2[1, 64:128] = SCALE
    return ind2


def _prep_inputs(x, w_qkv, w_out, b_out, g):
    xf = np.ascontiguousarray(x.reshape(DIM, N).astype(np.float32))
    w_q, w_k, w_v = (w_qkv[0:256], w_qkv[256:512], w_qkv[512:768])
    wqT = w_q.T.astype(np.float32)
    wkT = w_k.T.astype(np.float32)
    wvT = w_v.T.astype(np.float32)
    woT = w_out.T.astype(np.float32)
    wall = np.ascontiguousarray(np.stack(
        [wkT[0:128], wvT[0:128], wkT[128:256], wvT[128:256],
         wqT[0:128], wqT[128:256], woT[0:128], woT[128:256]], axis=1))
    common = {
        "wall": wall,
        "g": np.ascontiguousarray(g.astype(np.float32)),
        "onesr": np.ones((1, 128), np.float32),
        "br": np.ascontiguousarray(b_out.astype(np.float32).reshape(1, 256)),
        "ind2": _make_ind2(),
        "oc2": _make_oc2(),
        "bz": np.zeros((128, 128), np.float32),
    }
    in_maps = []
    for c in range(NCORES):
        m = dict(common)
        m["xt"] = np.ascontiguousarray(xf[:, c * TOK:(c + 1) * TOK])
        in_maps.append(m)
    return in_maps


def kernel(x, w_qkv, w_out, b_out, g):
    _install_bir_fix()
    from concourse.bass_utils import run_bass_kernel_spmd

    if "nc" not in _NC_CACHE:
        _NC_CACHE["nc"] = build_nc()
    nc = _NC_CACHE["nc"]
    in_maps = _prep_inputs(np.asarray(x), np.asarray(w_qkv), np.asarray(w_out),
                           np.asarray(b_out), np.asarray(g))
    res = run_bass_kernel_spmd(nc, in_maps, core_ids=list(range(NCORES)))
    y = np.empty((DIM, N), np.float32)
    for c in range(NCORES):
        y[:, c * TOK:(c + 1) * TOK] = res.results[c]["yt"].T
    return y.reshape(1, DIM, 16, 16, 16)


if __name__ == "__main__":
    import reference as R
    inputs = {k: np.asarray(v) for k, v in R.setup_inputs().items()}
    ref = np.asarray(R.reference(**inputs))
    got = kernel(**inputs)
    err = np.abs(got - ref)
    print("rel_absmax:", err.max() / np.abs(ref).max())
